# revision 6
# baseline (speedup 1.0000x reference)
"""Trainium2 Bass kernel for GaussianProcessEmbeddingHead.

The reference computes:
    mu     = x @ W_mu.T + b_mu                      (B,N,E)
    sigma  = exp(0.5*(x @ W_logvar.T + b_logvar))   (B,N,E)
    K      = RBF kernel matrix (B,N,N)  -- only its DIAGONAL is used,
             and dist_ii == 0 exactly, so cov_diag == 1 and the (B,N,N)
             work is mathematically dead. sigma_adjusted == sigma.
    return (mu, sigma_adjusted)

Strategy: data-parallel over batch B=8, one batch element per NeuronCore.
Per core: two linear heads over x_b [2048,1024] in bf16. The PE streams
one output column per cycle, so the floor is
   2 heads * (2048*512 outputs / 128 lanes) * (1024/128 k-tiles)
   = 131072 cycles ~= 54.6 us @ 2.4 GHz.

Schedule (v2): the old schedule idled the PE for ~9.5 us at the start
waiting for full-chunk SWDGE loads, and the warmup->stream gap (6.5 us)
re-throttled the HAM clock gate so the first ~12 real matmuls ran cold.
Now:
 - Critical-path loads are slab-granular on the two fast HWDGE queues:
   sync carries x chunk 0 in 4 pairs of k-tiles (256 KB each), scalar
   carries W_logvar in 4 matching pairs. First matmul issues ~2.4 us
   into exec; each kt-step consumes exactly one x slab + one w slab at
   the rate the rings deliver them.
 - Chunks run kt-OUTER / eb-inner with 4 PSUM banks accumulating in
   parallel, so compute needs only the k-slabs that have arrived, not
   the whole chunk. The final chunk reverts to eb-outer with a tapered
   last group (256/128/128) so the serialized end-of-kernel epilogue is
   short.
 - Warmup matmuls abut the real stream (PE busy continuously from
   ~0.2 us), so HAM flips to 2.4 GHz ~3.4 us in and stays there.
 - Remaining loads ride the free ring capacity: x_c1 split across
   sync+scalar right behind the critical slabs, x_c2/x_c3 + biases on
   gpsimd (SWDGE fixed cost hidden), wmu on scalar. lv stores ride
   sync, mu stores ride scalar.
 - Outputs are produced transposed ([E, N], partition = embedding), so
   each PSUM tile needs exactly ONE epilogue op with the bias fed
   through the per-partition port:
     sigma = Exp(PSUM * 0.5 + 0.5*b_lv[e])  on the Scalar engine
     mu    = PSUM + b_mu[e]                 on the Vector engine
   both writing bf16; host un-transposes and upcasts.
"""
import os
import sys

import numpy as np

try:
    import concourse.bass as bass  # noqa: F401
except Exception:  # pragma: no cover - path fallback for fresh dirs
    for p in ("/opt/trn_rl_repo", os.path.expanduser("~/.axon_site/_ro/trn_rl_repo")):
        if os.path.isdir(p) and p not in sys.path:
            sys.path.insert(0, p)
    import concourse.bass as bass

import ml_dtypes
import concourse.mybir as mybir
from concourse import bacc
from concourse.bass_utils import run_bass_kernel_spmd
from concourse.tile import TileContext

B, N, D, E = 8, 2048, 1024, 512
P = 128
KT = D // P          # 8 k-tiles
EB = E // P          # 4 embedding blocks
TC = N // 512        # 4 token chunks of 512
F32, BF16 = mybir.dt.float32, mybir.dt.bfloat16

_NC = None


def _build():
    nc = bacc.Bacc()
    # x packed on host as [p][c][kt][t] -> [P, KT*N]
    xP = nc.declare_dram_parameter("xP", [P, KT * N], BF16, isOutput=False)
    # weights packed as [p][kt][e] -> [P, KT*E]
    wlv = nc.declare_dram_parameter("wlv", [P, E * KT], BF16, isOutput=False)
    wmu = nc.declare_dram_parameter("wmu", [P, E * KT], BF16, isOutput=False)
    # biases arranged [P, EB]: element (p, eb) = bias[eb*128 + p]
    bmu = nc.declare_dram_parameter("bmu", [P, EB], F32, isOutput=False)
    blv = nc.declare_dram_parameter("blv", [P, EB], F32, isOutput=False)  # 0.5*b
    muT = nc.declare_dram_parameter("muT", [E, N], BF16, isOutput=True)
    sgT = nc.declare_dram_parameter("sgT", [E, N], BF16, isOutput=True)

    with TileContext(nc) as tc:
        with (
            tc.tile_pool(name="const", bufs=1) as cpool,
            tc.tile_pool(name="out", bufs=6) as opool,
            tc.tile_pool(name="psA", bufs=4, space="PSUM") as psA,
            tc.tile_pool(name="psB", bufs=4, space="PSUM") as psB,
        ):
            x_sb = [
                cpool.tile([P, KT, 512], BF16, name=f"x_sb{c}") for c in range(TC)
            ]
            wlv_sb = cpool.tile([P, KT, E], BF16)
            wmu_sb = cpool.tile([P, KT, E], BF16)
            blv_sb = cpool.tile([P, EB], F32)
            bmu_sb = cpool.tile([P, EB], F32)
            warm = cpool.tile([P, P], BF16)

            wlv_r = wlv[:, :].rearrange("p (kt e) -> p kt e", kt=KT)
            wmu_r = wmu[:, :].rearrange("p (kt e) -> p kt e", kt=KT)

            def xslab(c):
                off = c * 512 * KT
                return xP[:, off : off + 512 * KT].rearrange(
                    "p (kt t) -> p kt t", kt=KT
                )

            # Warmup: PE busy continuously from ~0.2us so the HAM clock
            # gate flips to 2.4 GHz ~3.4us in with no re-throttle gap.
            nc.vector.memset(warm, 0)
            wps = psA.tile([P, P], F32, tag="ps", name="warmps")
            for i in range(20):
                nc.tensor.matmul(
                    wps, warm[:, :], warm[:, :], start=(i == 0), stop=(i == 19)
                )

            # --- DMA schedule ---------------------------------------
            # The 3 DMA queues SHARE SDMA bandwidth (~270 GB/s aggregate
            # measured); a queue streaming non-critical data starves the
            # critical one. So phase the traffic:
            #  t=0-7us   sync: x_c0 in 4 kt-pairs | scalar: wlv in 4
            #            kt-pairs (the only traffic -> ~135 GB/s each,
            #            paced to the kt-outer consumption of chunk 0).
            #  t=7-18us  sync: x_c1 kt0-3, then lv stores | scalar:
            #            x_c1 kt4-7, wmu in 2 halves, then mu stores.
            #  t>9.5us   gpsimd: x_c2, x_c3 — gated behind chunk-0's
            #            first output tile (tiny copy below) so SWDGE
            #            cannot steal bandwidth during the lead-in; the
            #            interleaved head order defers their need to
            #            t~32/46us.
            for kp in range(4):
                nc.sync.dma_start(
                    out=x_sb[0][:, 2 * kp : 2 * kp + 2, :],
                    in_=xslab(0)[:, 2 * kp : 2 * kp + 2, :],
                )
                nc.scalar.dma_start(
                    out=wlv_sb[:, 2 * kp : 2 * kp + 2, :],
                    in_=wlv_r[:, 2 * kp : 2 * kp + 2, :],
                )
            nc.sync.dma_start(out=x_sb[1][:, 0:4, :], in_=xslab(1)[:, 0:4, :])
            nc.scalar.dma_start(out=x_sb[1][:, 4:KT, :], in_=xslab(1)[:, 4:KT, :])
            nc.scalar.dma_start(out=wmu_sb[:, 0:4, :], in_=wmu_r[:, 0:4, :])
            nc.scalar.dma_start(out=wmu_sb[:, 4:KT, :], in_=wmu_r[:, 4:KT, :])
            nc.gpsimd.dma_start(out=blv_sb, in_=blv[:, :])
            nc.gpsimd.dma_start(out=bmu_sb, in_=bmu[:, :])

            EXP = mybir.ActivationFunctionType.Exp

            def epilogue(hname, outdram, bias_sb, c, eb, ps, o0, ow):
                cs = slice(c * 512 + o0, c * 512 + o0 + ow)
                es = slice(eb * P, (eb + 1) * P)
                o = opool.tile([P, ow], BF16, tag="o", name=f"o_{hname}{c}{eb}_{o0}")
                if hname == "lv":
                    nc.scalar.activation(
                        o, ps, EXP, bias=bias_sb[:, eb : eb + 1], scale=0.5
                    )
                    nc.sync.dma_start(out=outdram[es, cs], in_=o)
                else:
                    nc.vector.tensor_scalar_add(o, ps, bias_sb[:, eb : eb + 1])
                    nc.scalar.dma_start(out=outdram[es, cs], in_=o)
                return o

            def chunk_ktouter(hname, w_sb, outdram, bias_sb, c, pool):
                """One token chunk, kt-outer: 4 PSUM banks accumulate in
                parallel; each kt step consumes one x slab + one w slab."""
                pss = [
                    pool.tile([P, 512], F32, tag="ps", name=f"ps_{hname}{c}{eb}")
                    for eb in range(EB)
                ]
                for kt in range(KT):
                    for eb in range(EB):
                        nc.tensor.matmul(
                            pss[eb],
                            w_sb[:, kt, eb * P : (eb + 1) * P],
                            x_sb[c][:, kt, :],
                            start=(kt == 0),
                            stop=(kt == KT - 1),
                        )
                outs = []
                for eb in range(EB):
                    outs.append(
                        epilogue(hname, outdram, bias_sb, c, eb, pss[eb], 0, 512)
                    )
                return outs

            def group_ebouter(hname, w_sb, outdram, bias_sb, c, eb, o0, ow, pool):
                """Baseline-style group: kt-inner over columns [o0:o0+ow)."""
                es = slice(eb * P, (eb + 1) * P)
                ps = pool.tile([P, ow], F32, tag="ps", name=f"ps_{hname}{c}{eb}_{o0}")
                for kt in range(KT):
                    nc.tensor.matmul(
                        ps,
                        w_sb[:, kt, es],
                        x_sb[c][:, kt, o0 : o0 + ow],
                        start=(kt == 0),
                        stop=(kt == KT - 1),
                    )
                epilogue(hname, outdram, bias_sb, c, eb, ps, o0, ow)

            LV = ("lv", wlv_sb, sgT, blv_sb)
            MU = ("mu", wmu_sb, muT, bmu_sb)
            # Interleaved head order: defers the x_c2/x_c3 need to
            # t~32us/46us so their loads stay off the critical window.
            order = [
                (LV, 0), (LV, 1), (MU, 0), (MU, 1),
                (LV, 2), (MU, 2), (LV, 3), (MU, 3),
            ]
            pools = [psA, psB]
            gate_sb = cpool.tile([P, 2], BF16)
            for ci, ((hname, w_sb, outdram, bias_sb), c) in enumerate(order):
                last = ci == len(order) - 1
                if not last:
                    outs = chunk_ktouter(
                        hname, w_sb, outdram, bias_sb, c, pools[ci % 2]
                    )
                    if ci == 0:
                        # Gate the gpsimd x_c2/x_c3 loads behind chunk-0's
                        # first output: the tiny copy makes the SWDGE
                        # issues wait until ~9.5us, keeping all SDMA
                        # bandwidth on the critical sync/scalar streams
                        # during the lead-in.
                        nc.gpsimd.tensor_copy(gate_sb, outs[0][:, 0:2])
                        nc.gpsimd.dma_start(out=x_sb[2], in_=xslab(2))
                        nc.gpsimd.dma_start(out=x_sb[3], in_=xslab(3))
                else:
                    # Final chunk: eb-outer with tapered last group so
                    # the serialized end-of-kernel epilogue is short.
                    pool = pools[ci % 2]
                    for eb in range(EB - 1):
                        group_ebouter(
                            hname, w_sb, outdram, bias_sb, c, eb, 0, 512, pool
                        )
                    for o0, ow in [(0, 256), (256, 128), (384, 128)]:
                        group_ebouter(
                            hname, w_sb, outdram, bias_sb, c, EB - 1, o0, ow, pool
                        )
    nc.compile()
    return nc


def _pack_x(xb):
    """xb [N, D] f32 -> [P, KT*N] bf16 packed as [p][c][kt][t]."""
    xt = xb.T.astype(ml_dtypes.bfloat16).reshape(KT, P, TC, 512)  # [kt, p, c, t]
    return np.ascontiguousarray(xt.transpose(1, 2, 0, 3).reshape(P, KT * N))


def _pack_w(W):
    """W [E, D] f32 -> [P, KT*E] bf16 packed as [p][kt][e]."""
    wt = W.astype(np.float32).T.astype(ml_dtypes.bfloat16)   # [D, E]
    v = wt.reshape(KT, P, E)
    return np.ascontiguousarray(v.transpose(1, 0, 2).reshape(P, KT * E))


def run(x, W_mu, b_mu, W_logvar, b_logvar, trace=False, **trace_kwargs):
    global _NC
    if _NC is None:
        _NC = _build()

    x = np.asarray(x, dtype=np.float32)
    wlv_h = _pack_w(np.asarray(W_logvar))
    wmu_h = _pack_w(np.asarray(W_mu))
    bmu_h = np.ascontiguousarray(np.asarray(b_mu, dtype=np.float32).reshape(EB, P).T)
    blv_h = np.ascontiguousarray(
        (0.5 * np.asarray(b_logvar, dtype=np.float32)).reshape(EB, P).T
    )

    in_maps = [
        {
            "xP": _pack_x(x[b]),
            "wlv": wlv_h,
            "wmu": wmu_h,
            "bmu": bmu_h,
            "blv": blv_h,
        }
        for b in range(B)
    ]
    res = run_bass_kernel_spmd(
        _NC, in_maps, core_ids=list(range(B)), trace=trace, **trace_kwargs
    )
    mu = np.stack(
        [res.results[b]["muT"].reshape(E, N).T.astype(np.float32) for b in range(B)]
    )
    sigma = np.stack(
        [res.results[b]["sgT"].reshape(E, N).T.astype(np.float32) for b in range(B)]
    )
    return (mu, sigma), res


def kernel(x, W_mu, b_mu, W_logvar, b_logvar):
    (mu, sigma), _ = run(x, W_mu, b_mu, W_logvar, b_logvar, trace=False)
    return mu, sigma


# revision 7
# speedup vs baseline: 1.0451x; 1.0451x over previous
"""Trainium2 Bass kernel for GaussianProcessEmbeddingHead.

The reference computes:
    mu     = x @ W_mu.T + b_mu                      (B,N,E)
    sigma  = exp(0.5*(x @ W_logvar.T + b_logvar))   (B,N,E)
    K      = RBF kernel matrix (B,N,N)  -- only its DIAGONAL is used,
             and dist_ii == 0 exactly, so cov_diag == 1 and the (B,N,N)
             work is mathematically dead. sigma_adjusted == sigma.
    return (mu, sigma_adjusted)

Strategy: data-parallel over batch B=8, one batch element per NeuronCore.
Per core: two linear heads over x_b [2048,1024] in bf16. The PE streams
one output column per cycle, so the floor is
   2 heads * (2048*512 outputs / 128 lanes) * (1024/128 k-tiles)
   = 131072 cycles ~= 54.6 us @ 2.4 GHz.

Schedule (v4) — built around two measured DMA facts:
 (1) queue throughput scales with the per-partition contiguous run
     (descriptor) size: ~45-90 GB/s at 1-2 KB, ~160+ at 4 KB, ~300 at
     8 KB; and concurrent queues share the SDMA fabric (~270 GB/s).
 (2) the old [128,512]-tile output stores were 1 KB descriptors; their
     drain clogged the queues to ~77 us and gated the kernel end.
So:
 - All bulk loads use >=4 KB descriptors: x chunks and weights move in
   half-chunk [128, 4kt] slabs on the two HWDGE queues (sync/scalar),
   x_c2/x_c3 as whole 8 KB-descriptor chunks on gpsimd.
 - Output DRAM layout is [p][eb][c][t] so stores are [128, 1024] spans
   (2 KB descriptors) issued per chunk-PAIR — half the store count,
   2-4x the rate, and they start at ~18 us instead of piling up late.
 - Compute runs head-major, kt-OUTER / eb-inner per chunk with 4 PSUM
   banks accumulating in parallel, so chunk 0 starts as soon as the
   first half-slabs land (~3.5 us) instead of waiting for full chunks.
 - gpsimd (SWDGE) loads are gated behind chunk-0's first output tile
   (tiny copy) so they cannot steal SDMA bandwidth during the lead-in.
 - Warmup matmuls abut the real stream so the HAM clock gate reaches
   2.4 GHz ~3.4 us in with no re-throttle gap.
 - The final chunk (mu c3) runs eb-outer with a tapered last group
   (256/128/128); each eb's [128,1024] store issues as that eb
   finishes, so the post-last-matmul tail is one 256 KB store.
 - Epilogues: one op per PSUM tile with the bias fed through the
   per-partition port:
     sigma = Exp(PSUM * 0.5 + 0.5*b_lv[e])  on the Scalar engine
     mu    = PSUM + b_mu[e]                 on the Vector engine
   both writing bf16; host un-transposes and upcasts.
"""
import os
import sys

import numpy as np

try:
    import concourse.bass as bass  # noqa: F401
except Exception:  # pragma: no cover - path fallback for fresh dirs
    for p in ("/opt/trn_rl_repo", os.path.expanduser("~/.axon_site/_ro/trn_rl_repo")):
        if os.path.isdir(p) and p not in sys.path:
            sys.path.insert(0, p)
    import concourse.bass as bass

import ml_dtypes
import concourse.mybir as mybir
from concourse import bacc
from concourse.bass_utils import run_bass_kernel_spmd
from concourse.tile import TileContext

B, N, D, E = 8, 2048, 1024, 512
P = 128
KT = D // P          # 8 k-tiles
EB = E // P          # 4 embedding blocks
TC = N // 512        # 4 token chunks of 512
F32, BF16 = mybir.dt.float32, mybir.dt.bfloat16

_NC = None


def _build():
    nc = bacc.Bacc()
    # x packed on host as [p][c][kt][t] -> [P, KT*N]
    xP = nc.declare_dram_parameter("xP", [P, KT * N], BF16, isOutput=False)
    # weights packed as [p][kt][e] -> [P, KT*E]
    wlv = nc.declare_dram_parameter("wlv", [P, E * KT], BF16, isOutput=False)
    wmu = nc.declare_dram_parameter("wmu", [P, E * KT], BF16, isOutput=False)
    # biases arranged [P, EB]: element (p, eb) = bias[eb*128 + p]
    bmu = nc.declare_dram_parameter("bmu", [P, EB], F32, isOutput=False)
    blv = nc.declare_dram_parameter("blv", [P, EB], F32, isOutput=False)  # 0.5*b
    # outputs packed [p][eb][c][t]: element (p, eb*N + c*512 + t) =
    # head[c*512 + t, eb*128 + p]
    muT = nc.declare_dram_parameter("muT", [P, EB * N], BF16, isOutput=True)
    sgT = nc.declare_dram_parameter("sgT", [P, EB * N], BF16, isOutput=True)

    with TileContext(nc) as tc:
        with (
            tc.tile_pool(name="const", bufs=1) as cpool,
            tc.tile_pool(name="out", bufs=8) as opool,
            tc.tile_pool(name="psA", bufs=4, space="PSUM") as psA,
            tc.tile_pool(name="psB", bufs=4, space="PSUM") as psB,
        ):
            x_sb = [
                cpool.tile([P, KT, 512], BF16, name=f"x_sb{c}") for c in range(TC)
            ]
            wlv_sb = cpool.tile([P, KT, E], BF16)
            wmu_sb = cpool.tile([P, KT, E], BF16)
            blv_sb = cpool.tile([P, EB], F32)
            bmu_sb = cpool.tile([P, EB], F32)
            warm = cpool.tile([P, P], BF16)
            gate_sb = cpool.tile([P, 2], BF16)

            wlv_r = wlv[:, :].rearrange("p (kt e) -> p kt e", kt=KT)
            wmu_r = wmu[:, :].rearrange("p (kt e) -> p kt e", kt=KT)

            def xslab(c):
                off = c * 512 * KT
                return xP[:, off : off + 512 * KT].rearrange(
                    "p (kt t) -> p kt t", kt=KT
                )

            # Warmup: PE busy continuously from ~0.2us so the HAM clock
            # gate flips to 2.4 GHz ~3.4us in with no re-throttle gap.
            nc.vector.memset(warm, 0)
            wps = psA.tile([P, P], F32, tag="ps", name="warmps")
            for i in range(20):
                nc.tensor.matmul(
                    wps, warm[:, :], warm[:, :], start=(i == 0), stop=(i == 19)
                )

            # --- bulk loads, all >=4KB descriptors ------------------
            # sync:   x_c0 h1, x_c0 h2, x_c1 h1, wmu h1, (lv stores)
            # scalar: wlv h1, wlv h2, x_c1 h2, wmu h2, (mu stores)
            # gpsimd: biases now; x_c2 + x_c3 gated at ~9.5us.
            nc.sync.dma_start(out=x_sb[0][:, 0:4, :], in_=xslab(0)[:, 0:4, :])
            nc.scalar.dma_start(out=wlv_sb[:, 0:4, :], in_=wlv_r[:, 0:4, :])
            nc.sync.dma_start(out=x_sb[0][:, 4:KT, :], in_=xslab(0)[:, 4:KT, :])
            nc.scalar.dma_start(out=wlv_sb[:, 4:KT, :], in_=wlv_r[:, 4:KT, :])
            nc.sync.dma_start(out=x_sb[1][:, 0:4, :], in_=xslab(1)[:, 0:4, :])
            nc.scalar.dma_start(out=x_sb[1][:, 4:KT, :], in_=xslab(1)[:, 4:KT, :])
            nc.sync.dma_start(out=wmu_sb[:, 0:4, :], in_=wmu_r[:, 0:4, :])
            nc.scalar.dma_start(out=wmu_sb[:, 4:KT, :], in_=wmu_r[:, 4:KT, :])
            nc.gpsimd.dma_start(out=blv_sb, in_=blv[:, :])
            nc.gpsimd.dma_start(out=bmu_sb, in_=bmu[:, :])

            EXP = mybir.ActivationFunctionType.Exp

            def epilogue(hname, bias_sb, eb, ps, ot, o0, ow):
                """PSUM -> bf16 slice [o0:o0+ow) of the [P,1024] out tile."""
                osl = ot[:, o0 : o0 + ow]
                if hname == "lv":
                    nc.scalar.activation(
                        osl, ps, EXP, bias=bias_sb[:, eb : eb + 1], scale=0.5
                    )
                else:
                    nc.vector.tensor_scalar_add(osl, ps, bias_sb[:, eb : eb + 1])

            def store(hname, outdram, eb, cp0, ot):
                """Store one [P,1024] out tile (chunks cp0, cp0+1 of eb)."""
                ds = slice(eb * N + cp0 * 512, eb * N + (cp0 + 2) * 512)
                if hname == "lv":
                    nc.sync.dma_start(out=outdram[:, ds], in_=ot)
                else:
                    nc.scalar.dma_start(out=outdram[:, ds], in_=ot)

            def chunk_ktouter(hname, w_sb, bias_sb, c, pool, otiles):
                """One token chunk, kt-outer: 4 PSUM banks accumulate in
                parallel; each kt step consumes one x half-slab."""
                pss = [
                    pool.tile([P, 512], F32, tag="ps", name=f"ps_{hname}{c}{eb}")
                    for eb in range(EB)
                ]
                for kt in range(KT):
                    for eb in range(EB):
                        nc.tensor.matmul(
                            pss[eb],
                            w_sb[:, kt, eb * P : (eb + 1) * P],
                            x_sb[c][:, kt, :],
                            start=(kt == 0),
                            stop=(kt == KT - 1),
                        )
                for eb in range(EB):
                    epilogue(hname, bias_sb, eb, pss[eb], otiles[eb], (c % 2) * 512, 512)

            pools = [psA, psB]
            for hi, (hname, w_sb, outdram, bias_sb) in enumerate(
                [("lv", wlv_sb, sgT, blv_sb), ("mu", wmu_sb, muT, bmu_sb)]
            ):
                for cp0 in (0, 2):
                    otiles = [
                        opool.tile([P, 1024], BF16, tag="o", name=f"o_{hname}{cp0}_{eb}")
                        for eb in range(EB)
                    ]
                    for c in (cp0, cp0 + 1):
                        ci = hi * TC + c
                        last_pass = hname == "mu" and c == TC - 1
                        pool = pools[ci % 2]
                        if not last_pass:
                            chunk_ktouter(hname, w_sb, bias_sb, c, pool, otiles)
                        else:
                            # Final chunk: eb-outer, tapered last group;
                            # each eb's [P,1024] store issues as soon as
                            # that eb finishes.
                            for eb in range(EB):
                                pieces = (
                                    [(0, 512)]
                                    if eb < EB - 1
                                    else [(0, 256), (256, 128), (384, 128)]
                                )
                                for o0, ow in pieces:
                                    ps = pool.tile(
                                        [P, ow], F32, tag="ps",
                                        name=f"ps_{hname}{c}{eb}_{o0}",
                                    )
                                    for kt in range(KT):
                                        nc.tensor.matmul(
                                            ps,
                                            w_sb[:, kt, eb * P : (eb + 1) * P],
                                            x_sb[c][:, kt, o0 : o0 + ow],
                                            start=(kt == 0),
                                            stop=(kt == KT - 1),
                                        )
                                    epilogue(
                                        hname, bias_sb, eb, ps, otiles[eb],
                                        512 + o0, ow,
                                    )
                                store(hname, outdram, eb, cp0, otiles[eb])
                        if hname == "lv" and c == 0:
                            # Gate the gpsimd x_c2/x_c3 loads behind
                            # chunk-0's first output tile: the tiny copy
                            # makes the SWDGE issues wait until ~9.5us so
                            # they can't steal SDMA bandwidth earlier.
                            nc.gpsimd.tensor_copy(gate_sb, otiles[0][:, 0:2])
                            nc.gpsimd.dma_start(out=x_sb[2], in_=xslab(2))
                            nc.gpsimd.dma_start(out=x_sb[3], in_=xslab(3))
                    if not (hname == "mu" and cp0 == 2):
                        for eb in range(EB):
                            store(hname, outdram, eb, cp0, otiles[eb])
    nc.compile()
    return nc


def _pack_x(xb):
    """xb [N, D] f32 -> [P, KT*N] bf16 packed as [p][c][kt][t]."""
    xt = xb.T.astype(ml_dtypes.bfloat16).reshape(KT, P, TC, 512)  # [kt, p, c, t]
    return np.ascontiguousarray(xt.transpose(1, 2, 0, 3).reshape(P, KT * N))


def _pack_w(W):
    """W [E, D] f32 -> [P, KT*E] bf16 packed as [p][kt][e]."""
    wt = W.astype(np.float32).T.astype(ml_dtypes.bfloat16)   # [D, E]
    v = wt.reshape(KT, P, E)
    return np.ascontiguousarray(v.transpose(1, 0, 2).reshape(P, KT * E))


def _unpack_out(a):
    """[P, EB*N] bf16 packed [p][eb][c][t] -> [N, E] f32."""
    v = a.reshape(P, EB, N)                      # [p, eb, n]
    return np.ascontiguousarray(v.transpose(2, 1, 0).reshape(N, E)).astype(np.float32)


def run(x, W_mu, b_mu, W_logvar, b_logvar, trace=False, **trace_kwargs):
    global _NC
    if _NC is None:
        _NC = _build()

    x = np.asarray(x, dtype=np.float32)
    wlv_h = _pack_w(np.asarray(W_logvar))
    wmu_h = _pack_w(np.asarray(W_mu))
    bmu_h = np.ascontiguousarray(np.asarray(b_mu, dtype=np.float32).reshape(EB, P).T)
    blv_h = np.ascontiguousarray(
        (0.5 * np.asarray(b_logvar, dtype=np.float32)).reshape(EB, P).T
    )

    in_maps = [
        {
            "xP": _pack_x(x[b]),
            "wlv": wlv_h,
            "wmu": wmu_h,
            "bmu": bmu_h,
            "blv": blv_h,
        }
        for b in range(B)
    ]
    res = run_bass_kernel_spmd(
        _NC, in_maps, core_ids=list(range(B)), trace=trace, **trace_kwargs
    )
    mu = np.stack([_unpack_out(res.results[b]["muT"]) for b in range(B)])
    sigma = np.stack([_unpack_out(res.results[b]["sgT"]) for b in range(B)])
    return (mu, sigma), res


def kernel(x, W_mu, b_mu, W_logvar, b_logvar):
    (mu, sigma), _ = run(x, W_mu, b_mu, W_logvar, b_logvar, trace=False)
    return mu, sigma
